# revision 8
# baseline (speedup 1.0000x reference)
"""APPNP GNN kernel for 8 Trainium2 NeuronCores.

Per core (nodes sharded, 12672 lanes incl. pads):
  MLP: X @ W0.T -> relu -> @ W1.T -> relu -> @ W2.T  (bf16 matmuls, f32 psum)
  K=10 propagation steps:
    hs = h * norm_src                      (bf16, [128, NT, 64])
    AllGather hs -> hbuf [NPAD, 64] bf16   (viewed as pair-table [NPAD/2, 128])
    dma_gather per (chunk, seg, half): 128B rows from the 256B-strided table
    one-hot S tiles (DVE is_equal vs iota) + S-stationary matmul segment-sum
    h = (1-a)*norm_dst*agg + a*h0
"""
import sys

sys.path.insert(0, "/opt/trn_rl_repo")

import numpy as np
import ml_dtypes

import inspect
import textwrap

import concourse.bass as bass
import concourse.bacc as bacc
import concourse.tile as tile
import concourse.mybir as mybir
from concourse.bass_utils import run_bass_kernel_spmd
from concourse.alu_op_type import AluOpType

BF16 = ml_dtypes.bfloat16
F32 = mybir.dt.float32
BF = mybir.dt.bfloat16
I16 = mybir.dt.int16

# problem constants
N = 100000
E = 1000000
IN = 512
C = 64
K = 10
ALPHA = 0.1

NCORES = 8
SH_N = N // NCORES              # real nodes per core
NTILES = (SH_N + 127) // 128 + 1  # node tiles per core (+1 all-pad tile)
SH = NTILES * 128               # padded nodes per core
NPAD = NCORES * SH
NSEG = 2                        # index segments (int16 range over pair table)
SEG_PAIRS = NPAD // 2 // NSEG   # pairs per segment
CHUNK_BANKS = 16                # banks per gather chunk
PAD_LANE_VAL = 200.0            # dst-lane sentinel for pad slots


def _core_seg_rows():
    return NPAD // NSEG  # rows per segment


def _install_dma_gather_patched():
    """Clone bass dma_gather with the 256B elem-size assert relaxed to 128B.

    The underlying ucode supports 128B transfers over a 256B-strided table;
    only the bass-side assert (written for the transpose path) blocks it.
    """
    if hasattr(bass.BassGpSimd, "dma_gather_patched"):
        return bass.BassGpSimd.dma_gather_patched
    src = inspect.getsource(bass.BassGpSimd.dma_gather)
    src = textwrap.dedent(src)
    src = src.replace(
        "elem_size_bytes > 0 and elem_size_bytes % 256 == 0",
        "elem_size_bytes > 0 and elem_size_bytes % 128 == 0",
    )
    src = src.replace("def dma_gather(", "def dma_gather_patched(")
    ns = dict(bass.BassGpSimd.dma_gather.__globals__)
    exec(compile(src, "<dma_gather_patched>", "exec"), ns)
    fn = ns["dma_gather_patched"]
    bass.BassGpSimd.dma_gather_patched = fn
    return fn


# ---------------------------------------------------------------------------
# host-side graph preprocessing
# ---------------------------------------------------------------------------

def preprocess(edge_index):
    src = np.asarray(edge_index[0], dtype=np.int64)
    dst = np.asarray(edge_index[1], dtype=np.int64)
    ne = src.shape[0]
    deg_out = np.bincount(src, minlength=N).astype(np.float32)
    deg_in = np.bincount(dst, minlength=N).astype(np.float32)
    ns_full = np.maximum(deg_out, 1.0) ** -0.5
    nd_full = np.maximum(deg_in, 1.0) ** -0.5

    core_of = np.arange(N) // SH_N
    lane_of = np.arange(N) % SH_N          # natural order within core
    p_of = lane_of % 128
    t_of = lane_of // 128
    # hbuf row (matches [128, NTILES, 64] (p, t, e) DMA layout)
    rr_of = core_of * SH + p_of * NTILES + t_of

    seg_rows = _core_seg_rows()
    nb = NTILES - 1  # active banks

    # edge -> (core, bank, class)
    c_e = core_of[dst]
    nt_e = t_of[dst]
    rr_s = rr_of[src]
    kappa_e = (rr_s // seg_rows) * 2 + (rr_s % 2)  # class 0..3

    # counts per (core, bank, class)
    key = ((c_e * nb + nt_e) * 4 + kappa_e).astype(np.int64)
    cnt = np.bincount(key, minlength=NCORES * nb * 4).reshape(NCORES, nb, 4)
    tk = -(-cnt // 128)              # tiles per (core, bank, class)
    TK = tk.max(axis=0)              # common schedule [nb, 4]

    # chunk structure over banks
    chunk_starts = list(range(0, nb, CHUNK_BANKS))
    chunks = [(s, min(s + CHUNK_BANKS, nb)) for s in chunk_starts]

    # column layout: for chunk: for kappa: for nt in chunk: TK[nt, kappa] tiles
    col_of = np.zeros((nb, 4), np.int64)   # first col of (nt, kappa)
    call_cols = []                          # per (chunk, kappa): (col0, ncols)
    col = 0
    for (b0, b1) in chunks:
        for kap in range(4):
            c0 = col
            for nt in range(b0, b1):
                col_of[nt, kap] = col
                col += TK[nt, kap]
            call_cols.append((c0, col - c0))
    ncols = col

    # pad target rows (zero hs) per class
    pad_pair_local = np.empty(4, np.int64)
    for kap in range(4):
        seg, half = kap // 2, kap % 2
        rr_pad = (seg * (NCORES // NSEG)) * SH + half * NTILES + (NTILES - 1)
        assert rr_pad % 2 == half and rr_pad // seg_rows == seg
        pad_pair_local[kap] = (rr_pad // 2) - seg * SEG_PAIRS

    # edge sort: by (core, bank, class)
    order_e = np.argsort(key, kind="stable")
    key_s = key[order_e]
    starts = np.concatenate([[0], np.cumsum(np.bincount(
        key_s, minlength=NCORES * nb * 4))])
    pos_e = np.arange(ne) - starts[key_s]

    src_sorted = src[order_e]
    dst_sorted = dst[order_e]
    c_s = c_e[order_e]
    nt_s = nt_e[order_e]
    kap_s = kappa_e[order_e]
    rr_ss = rr_of[src_sorted]
    seg_s = rr_ss // seg_rows
    pair_local_s = (rr_ss // 2) - seg_s * SEG_PAIRS
    lane_s = p_of[dst_sorted].astype(np.float32)

    gcol_s = col_of[nt_s, kap_s] + pos_e // 128
    gp_s = pos_e % 128

    colclass = np.empty(ncols, np.int64)
    for nt in range(nb):
        for kap in range(4):
            colclass[col_of[nt, kap]:col_of[nt, kap] + TK[nt, kap]] = kap

    idx_arrs = []   # [128, ncols] int16 (slot s at partition s)
    dl_arrs = []    # [128, ncols] f32 dst-lane per slot
    for c in range(NCORES):
        m = c_s == c
        idx_a = np.empty((128, ncols), np.int16)
        dl_a = np.full((128, ncols), PAD_LANE_VAL, np.float32)
        idx_a[:] = pad_pair_local[colclass][None, :].astype(np.int16)
        idx_a[gp_s[m], gcol_s[m]] = pair_local_s[m].astype(np.int16)
        dl_a[gp_s[m], gcol_s[m]] = lane_s[m]
        idx_arrs.append(idx_a)
        dl_arrs.append(dl_a)

    # wrapped gather index inputs: per call block [128, n_call/16]
    call_meta = []  # (kappa, col0, ncols_call, wrap_col0)
    wrap_col = 0
    for ci, (b0, b1) in enumerate(chunks):
        for kap in range(4):
            c0, ncol = call_cols[ci * 4 + kap]
            call_meta.append((kap, c0, ncol, wrap_col))
            wrap_col += ncol * 8  # 128 idx per col -> 8 wrap-cols
    totw = wrap_col
    idxw_arrs = []
    for c in range(NCORES):
        w = np.zeros((128, totw), np.int16)
        for (kap, c0, ncol, w0) in call_meta:
            if ncol == 0:
                continue
            blk = idx_arrs[c][:, c0:c0 + ncol]          # [128, ncol]
            flat = blk.T.reshape(-1)                    # slot order (col-major)
            wrapped = flat.reshape(-1, 16).T            # [16, ncol*8]
            w[:, w0:w0 + ncol * 8] = np.tile(wrapped, (8, 1))
        idxw_arrs.append(w)

    # norms [128, NTILES]
    ns_arrs, nd_arrs = [], []
    for c in range(NCORES):
        ns_a = np.zeros((128, NTILES), np.float32)
        nd_a = np.zeros((128, NTILES), np.float32)
        nodes = np.arange(c * SH_N, (c + 1) * SH_N)
        ns_a[p_of[nodes], t_of[nodes]] = ns_full[nodes]
        nd_a[p_of[nodes], t_of[nodes]] = (1.0 - ALPHA) * nd_full[nodes]
        ns_arrs.append(ns_a)
        nd_arrs.append(nd_a)

    iota = np.tile(np.arange(128, dtype=np.float32)[None, :], (128, 1))

    meta = dict(TK=TK, col_of=col_of, chunks=chunks, call_meta=call_meta,
                ncols=ncols, totw=totw)
    return dict(meta=meta, idxw_arrs=idxw_arrs, dl_arrs=dl_arrs,
                ns_arrs=ns_arrs, nd_arrs=nd_arrs, iota=iota,
                core_of=core_of, p_of=p_of, t_of=t_of)


# ---------------------------------------------------------------------------
# device graph builder
# ---------------------------------------------------------------------------

def build(meta):
    dma_gather_p = _install_dma_gather_patched()

    TK = meta["TK"]
    col_of = meta["col_of"]
    chunks = meta["chunks"]
    call_meta = meta["call_meta"]
    ncols = meta["ncols"]
    totw = meta["totw"]
    nb = NTILES - 1

    nc = bacc.Bacc("TRN2", target_bir_lowering=False, debug=False,
                   num_devices=NCORES)

    xt = nc.dram_tensor("xt", [128, 4, SH], BF, kind="ExternalInput")
    w0t = nc.dram_tensor("w0t", [128, 4, 512], BF, kind="ExternalInput")
    w1t = nc.dram_tensor("w1t", [128, 4, 256], BF, kind="ExternalInput")
    w2t = nc.dram_tensor("w2t", [128, 2, 64], BF, kind="ExternalInput")
    b0c = nc.dram_tensor("b0c", [128, 4], F32, kind="ExternalInput")
    b1c = nc.dram_tensor("b1c", [128, 2], F32, kind="ExternalInput")
    b2r = nc.dram_tensor("b2r", [128, 64], F32, kind="ExternalInput")
    nsb = nc.dram_tensor("nsb", [128, NTILES], F32, kind="ExternalInput")
    ndb = nc.dram_tensor("ndb", [128, NTILES], F32, kind="ExternalInput")
    iod = nc.dram_tensor("iota", [128, 128], F32, kind="ExternalInput")
    idxd = nc.dram_tensor("idxw", [128, totw], I16, kind="ExternalInput")
    dld = nc.dram_tensor("dlane", [128, ncols], F32, kind="ExternalInput")
    out = nc.dram_tensor("out", [128, NTILES, 64], F32, kind="ExternalOutput")

    max_chunk_cols = max(
        int(sum(TK[nt, kap] for nt in range(b0, b1) for kap in range(4)))
        for (b0, b1) in chunks)

    with tile.TileContext(nc) as tc:
        with (
            tc.tile_pool(name="const", bufs=1) as constp,
            tc.tile_pool(name="state", bufs=1) as statep,
            tc.tile_pool(name="xtp", bufs=3) as xtp,
            tc.tile_pool(name="a1p", bufs=2) as a1p,
            tc.tile_pool(name="a2p", bufs=2) as a2p,
            tc.tile_pool(name="msgp", bufs=2) as msgp,
            tc.tile_pool(name="sp", bufs=4) as spool,
            tc.tile_pool(name="ps_mlp", bufs=2, space="PSUM") as ps_mlp,
            tc.tile_pool(name="ps_prop", bufs=2, space="PSUM") as ps_prop,
            tc.tile_pool(name="dram", bufs=1, space="DRAM") as dramp,
        ):
            # ---- constants ----
            w0s = constp.tile([128, 4, 512], BF)
            w1s = constp.tile([128, 4, 256], BF)
            w2s = constp.tile([128, 2, 64], BF)
            b0s = constp.tile([128, 4], F32)
            b1s = constp.tile([128, 2], F32)
            b2s = constp.tile([128, 64], F32)
            nss = constp.tile([128, NTILES], F32)
            nds = constp.tile([128, NTILES], F32)
            ios = constp.tile([128, 128], F32)
            idxs = constp.tile([128, totw], I16)
            dls = constp.tile([128, ncols], F32)
            for dst_t, src_t in [(w0s, w0t), (w1s, w1t), (w2s, w2t),
                                 (b0s, b0c), (b1s, b1c), (b2s, b2r),
                                 (nss, nsb), (nds, ndb), (ios, iod),
                                 (idxs, idxd), (dls, dld)]:
                nc.sync.dma_start(dst_t[:], src_t[:])

            h_cur = statep.tile([128, NTILES, 64], F32)
            h0a = statep.tile([128, NTILES, 64], F32)
            hs = statep.tile([128, NTILES, 64], BF)

            # pad bank: zero once
            nc.vector.memset(h_cur[:, nb, :], 0.0)
            nc.vector.memset(h0a[:, nb, :], 0.0)
            nc.vector.memset(hs[:, nb, :], 0.0)

            # ---- MLP over chunks of 2 node-tiles ----
            for ch in range(nb // 2):
                c0 = ch * 256
                xt_t = xtp.tile([128, 4, 256], BF)
                nc.sync.dma_start(xt_t[:], xt[:, :, c0:c0 + 256])
                a1_t = a1p.tile([128, 4, 256], BF)
                for mt in range(4):
                    ps1 = ps_mlp.tile([128, 256], F32)
                    for k in range(4):
                        nc.tensor.matmul(
                            ps1[:],
                            lhsT=w0s[:, k, mt * 128:(mt + 1) * 128],
                            rhs=xt_t[:, k, :],
                            start=(k == 0), stop=(k == 3),
                        )
                    nc.scalar.activation(
                        a1_t[:, mt, :], ps1[:],
                        mybir.ActivationFunctionType.Relu,
                        bias=b0s[:, mt:mt + 1],
                    )
                a2_t = a2p.tile([128, 2, 256], BF)
                for mt in range(2):
                    ps2 = ps_mlp.tile([128, 256], F32)
                    for k in range(4):
                        nc.tensor.matmul(
                            ps2[:],
                            lhsT=w1s[:, k, mt * 128:(mt + 1) * 128],
                            rhs=a1_t[:, k, :],
                            start=(k == 0), stop=(k == 3),
                        )
                    nc.scalar.activation(
                        a2_t[:, mt, :], ps2[:],
                        mybir.ActivationFunctionType.Relu,
                        bias=b1s[:, mt:mt + 1],
                    )
                for sub in range(2):
                    nt = ch * 2 + sub
                    ps3 = ps_prop.tile([128, 64], F32)
                    for k in range(2):
                        nc.tensor.matmul(
                            ps3[:],
                            lhsT=a2_t[:, k, sub * 128:(sub + 1) * 128],
                            rhs=w2s[:, k, :],
                            start=(k == 0), stop=(k == 1),
                        )
                    nc.vector.tensor_add(h_cur[:, nt, :], ps3[:], b2s[:])
                    nc.vector.tensor_scalar_mul(
                        h0a[:, nt, :], h_cur[:, nt, :], ALPHA)
                    nc.scalar.activation(
                        hs[:, nt, :], h_cur[:, nt, :],
                        mybir.ActivationFunctionType.Identity,
                        scale=nss[:, nt:nt + 1],
                    )

            # ---- propagation ----
            cc_in = dramp.tile([128, NTILES, 64], BF)
            hbuf = dramp.tile([NPAD // 2, 128], BF)   # pair table

            for step in range(K):
                nc.sync.dma_start(cc_in[:], hs[:])
                nc.gpsimd.collective_compute(
                    "AllGather",
                    mybir.AluOpType.bypass,
                    replica_groups=[list(range(NCORES))],
                    ins=[cc_in.opt()],
                    outs=[hbuf.opt()],
                )
                for ci, (b0, b1) in enumerate(chunks):
                    ch_col0 = int(col_of[b0, 0])
                    msg_t = msgp.tile([128, max_chunk_cols, 64], BF)
                    for kap in range(4):
                        _, c0, ncol, w0 = call_meta[ci * 4 + kap]
                        if ncol == 0:
                            continue
                        seg, half = kap // 2, kap % 2
                        pr0 = seg * SEG_PAIRS
                        dma_gather_p(
                            nc.gpsimd,
                            out_ap=msg_t[:, c0 - ch_col0:c0 - ch_col0 + ncol, :],
                            in_ap=hbuf[pr0:pr0 + SEG_PAIRS,
                                       half * 64:(half + 1) * 64],
                            idxs_ap=idxs[:, w0:w0 + ncol * 8],
                            num_idxs=ncol * 128,
                            num_idxs_reg=ncol * 128,
                            elem_size=64,
                            elem_step=128,
                            single_packet=False,
                        )
                    for nt in range(b0, b1):
                        ntk = int(TK[nt].sum())
                        if ntk == 0:
                            nc.vector.tensor_scalar_mul(
                                h_cur[:, nt, :], h0a[:, nt, :], 1.0)
                        else:
                            ps = ps_prop.tile([128, 64], F32)
                            done = 0
                            for kap in range(4):
                                cb = int(col_of[nt, kap])
                                for g in range(int(TK[nt, kap])):
                                    col = cb + g
                                    s_t = spool.tile([128, 128], BF)
                                    nc.vector.tensor_scalar(
                                        s_t[:], ios[:], dls[:, col:col + 1],
                                        None, AluOpType.is_equal)
                                    nc.tensor.matmul(
                                        ps[:],
                                        lhsT=s_t[:],
                                        rhs=msg_t[:, col - ch_col0, :],
                                        start=(done == 0),
                                        stop=(done == ntk - 1),
                                    )
                                    done += 1
                            nc.vector.scalar_tensor_tensor(
                                h_cur[:, nt, :], ps[:], nds[:, nt:nt + 1],
                                h0a[:, nt, :],
                                AluOpType.mult, AluOpType.add,
                            )
                        if step < K - 1:
                            nc.scalar.activation(
                                hs[:, nt, :], h_cur[:, nt, :],
                                mybir.ActivationFunctionType.Identity,
                                scale=nss[:, nt:nt + 1],
                            )
            nc.sync.dma_start(out[:], h_cur[:])

    nc.compile()
    return nc


# ---------------------------------------------------------------------------
# host wrapper
# ---------------------------------------------------------------------------

def _prep_in_maps(features, W0, b0, W1, b1, W2, b2, pre):
    in_maps = []
    w0t = np.ascontiguousarray(
        W0.T.astype(BF16).reshape(4, 128, 512).transpose(1, 0, 2))
    w1t = np.ascontiguousarray(
        W1.T.astype(BF16).reshape(4, 128, 256).transpose(1, 0, 2))
    w2t = np.ascontiguousarray(
        W2.T.astype(BF16).reshape(2, 128, 64).transpose(1, 0, 2))
    b0cc = np.ascontiguousarray(b0.astype(np.float32).reshape(4, 128).T)
    b1cc = np.ascontiguousarray(b1.astype(np.float32).reshape(2, 128).T)
    b2rr = np.ascontiguousarray(
        np.tile(b2.astype(np.float32)[None, :], (128, 1)))
    X = features.astype(np.float32)
    for c in range(NCORES):
        nodes = np.arange(c * SH_N, (c + 1) * SH_N)
        xt_c = np.zeros((128, 4, SH), BF16)
        Xc = X[nodes].astype(BF16)
        xt_full = Xc.T.reshape(4, 128, SH_N).transpose(1, 0, 2)
        xt_c[:, :, :SH_N] = xt_full
        in_maps.append(dict(
            xt=xt_c, w0t=w0t, w1t=w1t, w2t=w2t,
            b0c=b0cc, b1c=b1cc, b2r=b2rr,
            nsb=pre["ns_arrs"][c], ndb=pre["nd_arrs"][c],
            iota=pre["iota"], idxw=pre["idxw_arrs"][c],
            dlane=pre["dl_arrs"][c],
        ))
    return in_maps


_CACHE = {}


def _get_compiled(edge_index):
    key = hash(np.asarray(edge_index).tobytes())
    if key not in _CACHE:
        pre = preprocess(edge_index)
        nc = build(pre["meta"])
        _CACHE[key] = (pre, nc)
    return _CACHE[key]


def kernel(features, edge_index, W0, b0, W1, b1, W2, b2, _trace=False):
    pre, nc = _get_compiled(edge_index)
    in_maps = _prep_in_maps(features, W0, b0, W1, b1, W2, b2, pre)
    res = run_bass_kernel_spmd(
        nc, in_maps, core_ids=list(range(NCORES)), trace=_trace)
    kernel.last_result = res
    out = np.empty((N, C), np.float32)
    p_of, t_of = pre["p_of"], pre["t_of"]
    for c in range(NCORES):
        nodes = np.arange(c * SH_N, (c + 1) * SH_N)
        oc = np.asarray(res.results[c]["out"]).reshape(128, NTILES, C)
        out[nodes] = oc[p_of[nodes], t_of[nodes]]
    return out


# revision 10
# speedup vs baseline: 295.3848x; 295.3848x over previous
"""APPNP GNN kernel for 8 Trainium2 NeuronCores.

Per core (nodes sharded, 12672 lanes incl. pads):
  MLP: X @ W0.T -> relu -> @ W1.T -> relu -> @ W2.T  (bf16 matmuls, f32 psum)
  K=10 propagation steps:
    hs = h * norm_src                      (bf16, [128, NT, 64])
    AllGather hs -> hbuf [NPAD, 64] bf16   (viewed as pair-table [NPAD/2, 128])
    dma_gather per (chunk, seg, half): 128B rows from the 256B-strided table
    one-hot S tiles (DVE is_equal vs iota) + S-stationary matmul segment-sum
    h = (1-a)*norm_dst*agg + a*h0
"""
import sys

sys.path.insert(0, "/opt/trn_rl_repo")

import numpy as np
import ml_dtypes

import inspect
import textwrap

import concourse.bass as bass
import concourse.bacc as bacc
import concourse.tile as tile
import concourse.mybir as mybir
from concourse.bass_utils import run_bass_kernel_spmd
from concourse.alu_op_type import AluOpType

BF16 = ml_dtypes.bfloat16
F32 = mybir.dt.float32
BF = mybir.dt.bfloat16
I16 = mybir.dt.int16

# problem constants
N = 100000
E = 1000000
IN = 512
C = 64
K = 10
ALPHA = 0.1

NCORES = 8
SH_N = N // NCORES              # real nodes per core
NTILES = (SH_N + 127) // 128 + 1  # node tiles per core (+1 all-pad tile)
SH = NTILES * 128               # padded nodes per core
NPAD = NCORES * SH
NSEG = 2                        # index segments (int16 range over pair table)
SEG_PAIRS = NPAD // 2 // NSEG   # pairs per segment
CHUNK_BANKS = 16                # banks per gather chunk
PAD_LANE_VAL = 200.0            # dst-lane sentinel for pad slots


def _core_seg_rows():
    return NPAD // NSEG  # rows per segment


def _install_dma_gather_patched():
    """Clone bass dma_gather with the 256B elem-size assert relaxed to 128B.

    The underlying ucode supports 128B transfers over a 256B-strided table;
    only the bass-side assert (written for the transpose path) blocks it.
    """
    if hasattr(bass.BassGpSimd, "dma_gather_patched"):
        return bass.BassGpSimd.dma_gather_patched
    src = inspect.getsource(bass.BassGpSimd.dma_gather)
    src = textwrap.dedent(src)
    src = src.replace(
        "elem_size_bytes > 0 and elem_size_bytes % 256 == 0",
        "elem_size_bytes > 0 and elem_size_bytes % 128 == 0",
    )
    src = src.replace("def dma_gather(", "def dma_gather_patched(")
    ns = dict(bass.BassGpSimd.dma_gather.__globals__)
    exec(compile(src, "<dma_gather_patched>", "exec"), ns)
    fn = ns["dma_gather_patched"]
    bass.BassGpSimd.dma_gather_patched = fn
    return fn


# ---------------------------------------------------------------------------
# host-side graph preprocessing
# ---------------------------------------------------------------------------

def preprocess(edge_index):
    src = np.asarray(edge_index[0], dtype=np.int64)
    dst = np.asarray(edge_index[1], dtype=np.int64)
    ne = src.shape[0]
    deg_out = np.bincount(src, minlength=N).astype(np.float32)
    deg_in = np.bincount(dst, minlength=N).astype(np.float32)
    ns_full = np.maximum(deg_out, 1.0) ** -0.5
    nd_full = np.maximum(deg_in, 1.0) ** -0.5

    core_of = np.arange(N) // SH_N
    lane_of = np.arange(N) % SH_N          # natural order within core
    p_of = lane_of % 128
    t_of = lane_of // 128
    # hbuf row (matches [128, NTILES, 64] (p, t, e) DMA layout)
    rr_of = core_of * SH + p_of * NTILES + t_of

    seg_rows = _core_seg_rows()
    nb = NTILES - 1  # active banks

    # edge -> (core, bank, class)
    c_e = core_of[dst]
    nt_e = t_of[dst]
    rr_s = rr_of[src]
    kappa_e = (rr_s // seg_rows) * 2 + (rr_s % 2)  # class 0..3

    # counts per (core, bank, class)
    key = ((c_e * nb + nt_e) * 4 + kappa_e).astype(np.int64)
    cnt = np.bincount(key, minlength=NCORES * nb * 4).reshape(NCORES, nb, 4)
    tk = -(-cnt // 128)              # tiles per (core, bank, class)
    TK = tk.max(axis=0)              # common schedule [nb, 4]

    # chunk structure over banks
    chunk_starts = list(range(0, nb, CHUNK_BANKS))
    chunks = [(s, min(s + CHUNK_BANKS, nb)) for s in chunk_starts]

    # column layout: for chunk: for kappa: for nt in chunk: TK[nt, kappa] tiles
    col_of = np.zeros((nb, 4), np.int64)   # first col of (nt, kappa)
    call_cols = []                          # per (chunk, kappa): (col0, ncols)
    col = 0
    for (b0, b1) in chunks:
        for kap in range(4):
            c0 = col
            for nt in range(b0, b1):
                col_of[nt, kap] = col
                col += TK[nt, kap]
            call_cols.append((c0, col - c0))
    ncols = col

    # pad target rows (zero hs) per class
    pad_pair_local = np.empty(4, np.int64)
    for kap in range(4):
        seg, half = kap // 2, kap % 2
        rr_pad = (seg * (NCORES // NSEG)) * SH + half * NTILES + (NTILES - 1)
        assert rr_pad % 2 == half and rr_pad // seg_rows == seg
        pad_pair_local[kap] = (rr_pad // 2) - seg * SEG_PAIRS

    # edge sort: by (core, bank, class)
    order_e = np.argsort(key, kind="stable")
    key_s = key[order_e]
    starts = np.concatenate([[0], np.cumsum(np.bincount(
        key_s, minlength=NCORES * nb * 4))])
    pos_e = np.arange(ne) - starts[key_s]

    src_sorted = src[order_e]
    dst_sorted = dst[order_e]
    c_s = c_e[order_e]
    nt_s = nt_e[order_e]
    kap_s = kappa_e[order_e]
    rr_ss = rr_of[src_sorted]
    seg_s = rr_ss // seg_rows
    pair_local_s = (rr_ss // 2) - seg_s * SEG_PAIRS
    lane_s = p_of[dst_sorted].astype(np.float32)

    gcol_s = col_of[nt_s, kap_s] + pos_e // 128
    gp_s = pos_e % 128

    colclass = np.empty(ncols, np.int64)
    for nt in range(nb):
        for kap in range(4):
            colclass[col_of[nt, kap]:col_of[nt, kap] + TK[nt, kap]] = kap

    idx_arrs = []   # [128, ncols] int16 (slot s at partition s)
    dl_arrs = []    # [128, ncols] f32 dst-lane per slot
    for c in range(NCORES):
        m = c_s == c
        idx_a = np.empty((128, ncols), np.int16)
        dl_a = np.full((128, ncols), PAD_LANE_VAL, np.float32)
        idx_a[:] = pad_pair_local[colclass][None, :].astype(np.int16)
        idx_a[gp_s[m], gcol_s[m]] = pair_local_s[m].astype(np.int16)
        dl_a[gp_s[m], gcol_s[m]] = lane_s[m]
        idx_arrs.append(idx_a)
        dl_arrs.append(dl_a)

    # wrapped gather index inputs: per call block [128, n_call/16]
    call_meta = []  # (kappa, col0, ncols_call, wrap_col0)
    wrap_col = 0
    for ci, (b0, b1) in enumerate(chunks):
        for kap in range(4):
            c0, ncol = call_cols[ci * 4 + kap]
            call_meta.append((kap, c0, ncol, wrap_col))
            wrap_col += ncol * 8  # 128 idx per col -> 8 wrap-cols
    totw = wrap_col
    idxw_arrs = []
    for c in range(NCORES):
        w = np.zeros((128, totw), np.int16)
        for (kap, c0, ncol, w0) in call_meta:
            if ncol == 0:
                continue
            blk = idx_arrs[c][:, c0:c0 + ncol]          # [128, ncol]
            flat = blk.T.reshape(-1)                    # slot order (col-major)
            wrapped = flat.reshape(-1, 16).T            # [16, ncol*8]
            w[:, w0:w0 + ncol * 8] = np.tile(wrapped, (8, 1))
        idxw_arrs.append(w)

    # norms [128, NTILES]
    ns_arrs, nd_arrs = [], []
    for c in range(NCORES):
        ns_a = np.zeros((128, NTILES), np.float32)
        nd_a = np.zeros((128, NTILES), np.float32)
        nodes = np.arange(c * SH_N, (c + 1) * SH_N)
        ns_a[p_of[nodes], t_of[nodes]] = ns_full[nodes]
        nd_a[p_of[nodes], t_of[nodes]] = (1.0 - ALPHA) * nd_full[nodes]
        ns_arrs.append(ns_a)
        nd_arrs.append(nd_a)

    iota = np.tile(np.arange(128, dtype=np.float32)[None, :], (128, 1))

    meta = dict(TK=TK, col_of=col_of, chunks=chunks, call_meta=call_meta,
                ncols=ncols, totw=totw)
    return dict(meta=meta, idxw_arrs=idxw_arrs, dl_arrs=dl_arrs,
                ns_arrs=ns_arrs, nd_arrs=nd_arrs, iota=iota,
                core_of=core_of, p_of=p_of, t_of=t_of)


# ---------------------------------------------------------------------------
# device graph builder
# ---------------------------------------------------------------------------

def build(meta, skip_cc=False, skip_gather=False, skip_mm=False):
    import os
    dma_gather_p = _install_dma_gather_patched()

    TK = meta["TK"]
    col_of = meta["col_of"]
    chunks = meta["chunks"]
    call_meta = meta["call_meta"]
    ncols = meta["ncols"]
    totw = meta["totw"]
    nb = NTILES - 1

    nc = bacc.Bacc("TRN2", target_bir_lowering=False, debug=False,
                   num_devices=NCORES)

    xt = nc.dram_tensor("xt", [128, 4, SH], BF, kind="ExternalInput")
    w0t = nc.dram_tensor("w0t", [128, 4, 512], BF, kind="ExternalInput")
    w1t = nc.dram_tensor("w1t", [128, 4, 256], BF, kind="ExternalInput")
    w2t = nc.dram_tensor("w2t", [128, 2, 64], BF, kind="ExternalInput")
    b0c = nc.dram_tensor("b0c", [128, 4], F32, kind="ExternalInput")
    b1c = nc.dram_tensor("b1c", [128, 2], F32, kind="ExternalInput")
    b2r = nc.dram_tensor("b2r", [128, 64], F32, kind="ExternalInput")
    nsb = nc.dram_tensor("nsb", [128, NTILES], F32, kind="ExternalInput")
    ndb = nc.dram_tensor("ndb", [128, NTILES], F32, kind="ExternalInput")
    iod = nc.dram_tensor("iota", [128, 128], F32, kind="ExternalInput")
    idxd = nc.dram_tensor("idxw", [128, totw], I16, kind="ExternalInput")
    dld = nc.dram_tensor("dlane", [128, ncols], F32, kind="ExternalInput")
    out = nc.dram_tensor("out", [128, NTILES, 64], F32, kind="ExternalOutput")

    max_chunk_cols = max(
        int(sum(TK[nt, kap] for nt in range(b0, b1) for kap in range(4)))
        for (b0, b1) in chunks)

    with tile.TileContext(nc) as tc:
        with (
            tc.tile_pool(name="const", bufs=1) as constp,
            tc.tile_pool(name="state", bufs=1) as statep,
            tc.tile_pool(name="xtp", bufs=3) as xtp,
            tc.tile_pool(name="a1p", bufs=2) as a1p,
            tc.tile_pool(name="a2p", bufs=2) as a2p,
            tc.tile_pool(name="msgp", bufs=2) as msgp,
            tc.tile_pool(name="sp", bufs=4) as spool,
            tc.tile_pool(name="ps_mlp", bufs=2, space="PSUM") as ps_mlp,
            tc.tile_pool(name="ps_prop", bufs=2, space="PSUM") as ps_prop,
            tc.tile_pool(name="dram", bufs=1, space="DRAM") as dramp,
        ):
            # ---- constants ----
            w0s = constp.tile([128, 4, 512], BF)
            w1s = constp.tile([128, 4, 256], BF)
            w2s = constp.tile([128, 2, 64], BF)
            b0s = constp.tile([128, 4], F32)
            b1s = constp.tile([128, 2], F32)
            b2s = constp.tile([128, 64], F32)
            nss = constp.tile([128, NTILES], F32)
            nds = constp.tile([128, NTILES], F32)
            ios = constp.tile([128, 128], F32)
            idxs = constp.tile([128, totw], I16)
            dls = constp.tile([128, ncols], F32)
            for dst_t, src_t in [(w0s, w0t), (w1s, w1t), (w2s, w2t),
                                 (b0s, b0c), (b1s, b1c), (b2s, b2r),
                                 (nss, nsb), (nds, ndb), (ios, iod),
                                 (idxs, idxd), (dls, dld)]:
                nc.sync.dma_start(dst_t[:], src_t[:])

            h_cur = statep.tile([128, NTILES, 64], F32)
            h0a = statep.tile([128, NTILES, 64], F32)
            hs = statep.tile([128, NTILES, 64], BF)

            # pad bank: zero once
            nc.vector.memset(h_cur[:, nb, :], 0.0)
            nc.vector.memset(h0a[:, nb, :], 0.0)
            nc.vector.memset(hs[:, nb, :], 0.0)

            # ---- MLP over chunks of 2 node-tiles ----
            for ch in range(nb // 2):
                c0 = ch * 256
                xt_t = xtp.tile([128, 4, 256], BF)
                nc.sync.dma_start(xt_t[:], xt[:, :, c0:c0 + 256])
                a1_t = a1p.tile([128, 4, 256], BF)
                for mt in range(4):
                    ps1 = ps_mlp.tile([128, 256], F32)
                    for k in range(4):
                        nc.tensor.matmul(
                            ps1[:],
                            lhsT=w0s[:, k, mt * 128:(mt + 1) * 128],
                            rhs=xt_t[:, k, :],
                            start=(k == 0), stop=(k == 3),
                        )
                    nc.scalar.activation(
                        a1_t[:, mt, :], ps1[:],
                        mybir.ActivationFunctionType.Relu,
                        bias=b0s[:, mt:mt + 1],
                    )
                a2_t = a2p.tile([128, 2, 256], BF)
                for mt in range(2):
                    ps2 = ps_mlp.tile([128, 256], F32)
                    for k in range(4):
                        nc.tensor.matmul(
                            ps2[:],
                            lhsT=w1s[:, k, mt * 128:(mt + 1) * 128],
                            rhs=a1_t[:, k, :],
                            start=(k == 0), stop=(k == 3),
                        )
                    nc.scalar.activation(
                        a2_t[:, mt, :], ps2[:],
                        mybir.ActivationFunctionType.Relu,
                        bias=b1s[:, mt:mt + 1],
                    )
                for sub in range(2):
                    nt = ch * 2 + sub
                    ps3 = ps_prop.tile([128, 64], F32)
                    for k in range(2):
                        nc.tensor.matmul(
                            ps3[:],
                            lhsT=a2_t[:, k, sub * 128:(sub + 1) * 128],
                            rhs=w2s[:, k, :],
                            start=(k == 0), stop=(k == 1),
                        )
                    nc.vector.tensor_add(h_cur[:, nt, :], ps3[:], b2s[:])
                    nc.vector.tensor_scalar_mul(
                        h0a[:, nt, :], h_cur[:, nt, :], ALPHA)
                    nc.scalar.activation(
                        hs[:, nt, :], h_cur[:, nt, :],
                        mybir.ActivationFunctionType.Identity,
                        scale=nss[:, nt:nt + 1],
                    )

            # ---- propagation ----
            cc_in = dramp.tile([128, NTILES, 64], BF)
            hbuf = dramp.tile([NPAD // 2, 128], BF)   # pair table

            for step in range(K):
                nc.sync.dma_start(cc_in[:], hs[:])
                if not skip_cc:
                    nc.gpsimd.collective_compute(
                        "AllGather",
                        mybir.AluOpType.bypass,
                        replica_groups=[list(range(NCORES))],
                        ins=[cc_in.opt()],
                        outs=[hbuf.opt()],
                    )
                for ci, (b0, b1) in enumerate(chunks):
                    ch_col0 = int(col_of[b0, 0])
                    msg_t = msgp.tile([128, max_chunk_cols, 64], BF)
                    for kap in range(4):
                        _, c0, ncol, w0 = call_meta[ci * 4 + kap]
                        if ncol == 0 or skip_gather:
                            continue
                        seg, half = kap // 2, kap % 2
                        pr0 = seg * SEG_PAIRS
                        # single_packet packs 64 descs/engine -> <=1024 idx
                        for sc in range(0, ncol, 8):
                            sn = min(8, ncol - sc)
                            o0 = c0 - ch_col0 + sc
                            dma_gather_p(
                                nc.gpsimd,
                                out_ap=msg_t[:, o0:o0 + sn, :],
                                in_ap=hbuf[pr0:pr0 + SEG_PAIRS,
                                           half * 64:(half + 1) * 64],
                                idxs_ap=idxs[:, w0 + sc * 8:w0 + (sc + sn) * 8],
                                num_idxs=sn * 128,
                                num_idxs_reg=sn * 128,
                                elem_size=64,
                                elem_step=128,
                                single_packet=True,
                            )
                    for nt in range(b0, b1):
                        ntk = int(TK[nt].sum())
                        if ntk == 0:
                            nc.vector.tensor_scalar_mul(
                                h_cur[:, nt, :], h0a[:, nt, :], 1.0)
                        else:
                            ps = ps_prop.tile([128, 64], F32)
                            done = 0
                            for kap in range(4):
                                cb = int(col_of[nt, kap])
                                for g in range(int(TK[nt, kap])):
                                    col = cb + g
                                    s_t = spool.tile([128, 128], BF)
                                    nc.vector.tensor_scalar(
                                        s_t[:], ios[:], dls[:, col:col + 1],
                                        None, AluOpType.is_equal)
                                    if not skip_mm or done == 0 or done == ntk - 1:
                                        nc.tensor.matmul(
                                            ps[:],
                                            lhsT=s_t[:],
                                            rhs=msg_t[:, col - ch_col0, :],
                                            start=(done == 0),
                                            stop=(done == ntk - 1),
                                        )
                                    done += 1
                            nc.vector.scalar_tensor_tensor(
                                h_cur[:, nt, :], ps[:], nds[:, nt:nt + 1],
                                h0a[:, nt, :],
                                AluOpType.mult, AluOpType.add,
                            )
                        if step < K - 1:
                            nc.scalar.activation(
                                hs[:, nt, :], h_cur[:, nt, :],
                                mybir.ActivationFunctionType.Identity,
                                scale=nss[:, nt:nt + 1],
                            )
            nc.sync.dma_start(out[:], h_cur[:])

    nc.compile()
    return nc


# ---------------------------------------------------------------------------
# host wrapper
# ---------------------------------------------------------------------------

def _prep_in_maps(features, W0, b0, W1, b1, W2, b2, pre):
    in_maps = []
    w0t = np.ascontiguousarray(
        W0.T.astype(BF16).reshape(4, 128, 512).transpose(1, 0, 2))
    w1t = np.ascontiguousarray(
        W1.T.astype(BF16).reshape(4, 128, 256).transpose(1, 0, 2))
    w2t = np.ascontiguousarray(
        W2.T.astype(BF16).reshape(2, 128, 64).transpose(1, 0, 2))
    b0cc = np.ascontiguousarray(b0.astype(np.float32).reshape(4, 128).T)
    b1cc = np.ascontiguousarray(b1.astype(np.float32).reshape(2, 128).T)
    b2rr = np.ascontiguousarray(
        np.tile(b2.astype(np.float32)[None, :], (128, 1)))
    X = features.astype(np.float32)
    for c in range(NCORES):
        nodes = np.arange(c * SH_N, (c + 1) * SH_N)
        xt_c = np.zeros((128, 4, SH), BF16)
        Xc = X[nodes].astype(BF16)
        xt_full = Xc.T.reshape(4, 128, SH_N).transpose(1, 0, 2)
        xt_c[:, :, :SH_N] = xt_full
        in_maps.append(dict(
            xt=xt_c, w0t=w0t, w1t=w1t, w2t=w2t,
            b0c=b0cc, b1c=b1cc, b2r=b2rr,
            nsb=pre["ns_arrs"][c], ndb=pre["nd_arrs"][c],
            iota=pre["iota"], idxw=pre["idxw_arrs"][c],
            dlane=pre["dl_arrs"][c],
        ))
    return in_maps


_CACHE = {}


def _get_compiled(edge_index):
    key = hash(np.asarray(edge_index).tobytes())
    if key not in _CACHE:
        pre = preprocess(edge_index)
        nc = build(pre["meta"])
        _CACHE[key] = (pre, nc)
    return _CACHE[key]


def kernel(features, edge_index, W0, b0, W1, b1, W2, b2, _trace=False):
    pre, nc = _get_compiled(edge_index)
    in_maps = _prep_in_maps(features, W0, b0, W1, b1, W2, b2, pre)
    res = run_bass_kernel_spmd(
        nc, in_maps, core_ids=list(range(NCORES)), trace=_trace)
    kernel.last_result = res
    out = np.empty((N, C), np.float32)
    p_of, t_of = pre["p_of"], pre["t_of"]
    for c in range(NCORES):
        nodes = np.arange(c * SH_N, (c + 1) * SH_N)
        oc = np.asarray(res.results[c]["out"]).reshape(128, NTILES, C)
        out[nodes] = oc[p_of[nodes], t_of[nodes]]
    return out
